# revision 3
# baseline (speedup 1.0000x reference)
"""Trainium2 Bass kernel for nn_CutoffModule (CBAM-style channel gate + topk gather).

Reference computation (per sample):
    avg/max spatial pooling -> shared 2-layer MLP -> sum -> sigmoid -> attn [C, D]
    per scale d: top-128 channels (sorted desc) -> gather those channels of x.

Sharding: data-parallel over N across 8 cores (4 samples/core); MLP weights
replicated. Entirely self-contained: hardcodes N=32, C=512, H=W=64, D=4, r=16.

Note: sigmoid is strictly monotonic, so top_k(sigmoid(y)) == top_k(y); the
kernel ranks pre-sigmoid logits and never materializes the sigmoid.
"""

import numpy as np

import concourse.bacc as bacc
import concourse.bass as bass
import concourse.tile as tile
from concourse import mybir
from concourse.bass_utils import run_bass_kernel_spmd

# Problem constants (hardcoded per harness contract)
N_FULL = 32
C = 512
HW = 64 * 64          # 4096
D = 4                 # depth scales
BLOCK = C // D        # 128
HID = C // 16         # 32  (MLP hidden)
N_CORES = 8
NS = N_FULL // N_CORES  # 4 samples per core
P = 128               # SBUF partitions
CT = C // P           # 4 channel tiles per sample
ROWS = NS * D         # 16 topk rows per core (row r = d*4 + n)
NEG_FILL = -1e30

F32 = mybir.dt.float32
U32 = mybir.dt.uint32


def _build_program():
    nc = bacc.Bacc("TRN2", target_bir_lowering=False, debug=False)

    x_d = nc.dram_tensor("x", [NS * C, HW], F32, kind="ExternalInput").ap()
    w1_d = nc.dram_tensor("w1", [C, HID], F32, kind="ExternalInput").ap()
    b1_d = nc.dram_tensor("b1", [HID, 1], F32, kind="ExternalInput").ap()
    w2_d = nc.dram_tensor("w2", [HID, C * D], F32, kind="ExternalInput").ap()
    b2_d = nc.dram_tensor("b2", [1, C * D], F32, kind="ExternalInput").ap()
    ident_d = nc.dram_tensor("ident", [ROWS, ROWS], F32, kind="ExternalInput").ap()
    nofs_d = nc.dram_tensor("nofs", [ROWS, 1], F32, kind="ExternalInput").ap()
    out_d = nc.dram_tensor("out", [NS * C, HW], F32, kind="ExternalOutput").ap()

    with tile.TileContext(nc) as tc:
        with (
            tc.tile_pool(name="xin", bufs=3) as xin_pool,
            tc.tile_pool(name="gbuf", bufs=3) as g_pool,
            tc.tile_pool(name="small", bufs=1) as sm,
            tc.tile_pool(name="psum", bufs=1, space="PSUM") as psum,
        ):
            # ---- constants / weights into SBUF ----
            w1_sb = sm.tile([P, CT, HID], F32)   # chunk ct = channels ct*128..+128
            nc.sync.dma_start(
                out=w1_sb[:], in_=w1_d.rearrange("(c p) m -> p c m", p=P)
            )
            w2_sb = sm.tile([HID, C * D], F32)
            nc.sync.dma_start(out=w2_sb[:], in_=w2_d)
            b1_sb = sm.tile([HID, 1], F32)
            nc.sync.dma_start(out=b1_sb[:], in_=b1_d)
            b2_sb = sm.tile([1, C * D], F32)
            nc.sync.dma_start(out=b2_sb[:], in_=b2_d)
            ident_sb = sm.tile([ROWS, ROWS], F32)
            nc.sync.dma_start(out=ident_sb[:], in_=ident_d)
            nofs_sb = sm.tile([ROWS, 1], F32)
            nc.sync.dma_start(out=nofs_sb[:], in_=nofs_d)
            twos_sb = sm.tile([1, NS], F32)
            nc.vector.memset(twos_sb[:], 2.0)

            # ---- pass 1: pooling over spatial dim ----
            # avgp/maxp layout: [P, ct, n] -> matmul rhs slice [:, ct, :] = [K=128, NS]
            avgp = sm.tile([P, CT, NS], F32)
            maxp = sm.tile([P, CT, NS], F32)
            scratch = sm.tile([P, HW], F32)
            for n in range(NS):
                for ct in range(CT):
                    row0 = (n * CT + ct) * P
                    xt = xin_pool.tile([P, HW], F32, tag="xt")
                    nc.sync.dma_start(out=xt[:], in_=x_d[row0 : row0 + P, :])
                    # avg pool on ScalarE: accum_out sums copy(x * 1/HW)
                    nc.scalar.activation(
                        out=scratch[:],
                        in_=xt[:],
                        func=mybir.ActivationFunctionType.Copy,
                        scale=1.0 / HW,
                        accum_out=avgp[:, ct, n : n + 1],
                    )
                    nc.vector.reduce_max(
                        out=maxp[:, ct, n : n + 1], in_=xt[:], axis=mybir.AxisListType.X
                    )

            # ---- MLP: y = mlp(avg) + mlp(max) + 2*b2 (b1 inside relu) ----
            # layer 1 computes h^T = W1^T @ p^T  -> [HID, NS]; b1 is a per-partition
            # bias applied by the Relu activation.
            h_sb = {}
            for name, pool_t in (("a", avgp), ("m", maxp)):
                ph = psum.tile([HID, NS], F32, space="PSUM", tag="ph")
                for ct in range(CT):
                    nc.tensor.matmul(
                        out=ph[:],
                        lhsT=w1_sb[:, ct, :],
                        rhs=pool_t[:, ct, :],
                        start=(ct == 0),
                        stop=(ct == CT - 1),
                    )
                hT = sm.tile([HID, NS], F32, name=f"hT_{name}")
                nc.scalar.activation(
                    out=hT[:],
                    in_=ph[:],
                    func=mybir.ActivationFunctionType.Relu,
                    bias=b1_sb[:, :],
                )
                h_sb[name] = hT

            # layer 2: y[n, :] accumulated in one PSUM group per 512-wide slice:
            # h_a@W2 + h_m@W2 + 2*b2  (reference adds b2 once per mlp call)
            py = psum.tile([NS, C * D], F32, space="PSUM", tag="py")
            for s in range(C * D // 512):
                sl = slice(s * 512, (s + 1) * 512)
                nc.tensor.matmul(
                    out=py[:, sl], lhsT=h_sb["a"][:], rhs=w2_sb[:, sl],
                    start=True, stop=False,
                )
                nc.tensor.matmul(
                    out=py[:, sl], lhsT=h_sb["m"][:], rhs=w2_sb[:, sl],
                    start=False, stop=False,
                )
                nc.tensor.matmul(
                    out=py[:, sl], lhsT=twos_sb[:], rhs=b2_sb[:, sl],
                    start=False, stop=True,
                )

            # ---- topk rows: vals[r= d*4+n, c] = y[n, c*D + d] ----
            # DVE writes must start at partition 0/32/64/96, so evacuate PSUM
            # once and let DMA (partition-agnostic) place the strided rows.
            y_sb = sm.tile([NS, C * D], F32)
            nc.vector.tensor_copy(out=y_sb[:], in_=py[:])
            vals_a = sm.tile([ROWS, C], F32)
            vals_b = sm.tile([ROWS, C], F32)
            for d in range(D):
                nc.sync.dma_start(
                    out=vals_a[4 * d : 4 * d + NS, :],
                    in_=y_sb[:, d :: D],
                )

            topk_idx = sm.tile([ROWS, BLOCK], U32)
            maxv = sm.tile([ROWS, 8], F32)
            cur, nxt = vals_a, vals_b
            for k in range(BLOCK // 8):
                nc.vector.max(out=maxv[:], in_=cur[:])
                nc.vector.max_index(
                    out=topk_idx[:, 8 * k : 8 * k + 8], in_max=maxv[:], in_values=cur[:]
                )
                if k < BLOCK // 8 - 1:
                    nc.vector.match_replace(
                        out=nxt[:], in_to_replace=maxv[:], in_values=cur[:],
                        imm_value=NEG_FILL,
                    )
                    cur, nxt = nxt, cur

            # ---- indices: +n*512 (DRAM row base), transpose to per-partition cols ----
            idx_f = sm.tile([ROWS, BLOCK], F32)
            nc.vector.tensor_copy(out=idx_f[:], in_=topk_idx[:])
            nc.vector.tensor_scalar_add(idx_f[:], idx_f[:], nofs_sb[:, :])
            pt = psum.tile([BLOCK, ROWS], F32, space="PSUM", tag="pt")
            nc.tensor.transpose(out=pt[:], in_=idx_f[:], identity=ident_sb[:])
            idxT = sm.tile([BLOCK, ROWS], U32)
            nc.vector.tensor_copy(out=idxT[:], in_=pt[:])

            # ---- gather + store per (n, d) ----
            for n in range(NS):
                for d in range(D):
                    r = 4 * d + n
                    g = g_pool.tile([P, HW], F32, tag="g")
                    nc.gpsimd.indirect_dma_start(
                        out=g[:],
                        out_offset=None,
                        in_=x_d[:, :],
                        in_offset=bass.IndirectOffsetOnAxis(
                            ap=idxT[:, r : r + 1], axis=0
                        ),
                    )
                    o0 = n * C + d * BLOCK
                    nc.scalar.dma_start(out=out_d[o0 : o0 + BLOCK, :], in_=g[:])

    nc.compile()
    return nc


_NC_CACHE = None


def _get_nc():
    global _NC_CACHE
    if _NC_CACHE is None:
        _NC_CACHE = _build_program()
    return _NC_CACHE


def _make_in_maps(x, W1, b1, W2, b2):
    x = np.ascontiguousarray(np.asarray(x, dtype=np.float32)).reshape(N_FULL, C, HW)
    W1 = np.asarray(W1, dtype=np.float32)
    b1 = np.asarray(b1, dtype=np.float32).reshape(HID, 1)
    W2 = np.asarray(W2, dtype=np.float32)
    b2 = np.asarray(b2, dtype=np.float32).reshape(1, C * D)
    ident = np.eye(ROWS, dtype=np.float32)
    nofs = ((np.arange(ROWS, dtype=np.float32) % NS) * C).reshape(ROWS, 1)
    in_maps = []
    for core in range(N_CORES):
        shard = x[core * NS : (core + 1) * NS].reshape(NS * C, HW)
        in_maps.append(
            {
                "x": np.ascontiguousarray(shard),
                "w1": W1,
                "b1": b1,
                "w2": W2,
                "b2": b2,
                "ident": ident,
                "nofs": nofs,
            }
        )
    return in_maps


def run(inputs, trace=False, **kwargs):
    """Run the SPMD kernel; returns (full_output, BassKernelResults)."""
    nc = _get_nc()
    in_maps = _make_in_maps(
        inputs["x"], inputs["W1"], inputs["b1"], inputs["W2"], inputs["b2"]
    )
    res = run_bass_kernel_spmd(
        nc, in_maps, core_ids=list(range(N_CORES)), trace=trace, **kwargs
    )
    parts = [res.results[i]["out"].reshape(NS, C, 64, 64) for i in range(N_CORES)]
    out = np.concatenate(parts, axis=0)
    return out, res


def kernel(**inputs) -> np.ndarray:
    out, _ = run(inputs)
    return out


# revision 6
# speedup vs baseline: 1.2023x; 1.2023x over previous
"""Trainium2 Bass kernel for nn_CutoffModule (CBAM-style channel gate + topk gather).

Reference computation (per sample):
    avg/max spatial pooling -> shared 2-layer MLP -> sum -> sigmoid -> attn [C, D]
    per scale d: top-128 channels (sorted desc) -> gather those channels of x.

Sharding: data-parallel over N across 8 cores (4 samples/core); MLP weights
replicated. Entirely self-contained: hardcodes N=32, C=512, H=W=64, D=4, r=16.

Notes:
- sigmoid is strictly monotonic, so top_k(sigmoid(y)) == top_k(y); the kernel
  ranks pre-sigmoid logits and never materializes the sigmoid.
- topk row (d, n) lives on SBUF partition 32*d + n: engine writes must start
  at partition 0/32/64/96, and this layout lets plain DVE copies slice
  y[n, d::4] out of PSUM with legal partition offsets.
- samples are processed in two pairs so the gather+store DMA of pair 0
  overlaps the MLP+topk of pair 1.
"""

import numpy as np

import concourse.bacc as bacc
import concourse.bass as bass
import concourse.tile as tile
from concourse import mybir
from concourse.bass_utils import run_bass_kernel_spmd

# Problem constants (hardcoded per harness contract)
N_FULL = 32
C = 512
HW = 64 * 64          # 4096
D = 4                 # depth scales
BLOCK = C // D        # 128
HID = C // 16         # 32  (MLP hidden)
N_CORES = 8
NS = N_FULL // N_CORES  # 4 samples per core
P = 128               # SBUF partitions
CT = C // P           # 4 channel tiles per sample
NEG_FILL = -1e30

F32 = mybir.dt.float32
U32 = mybir.dt.uint32


def _build_program():
    nc = bacc.Bacc("TRN2", target_bir_lowering=False, debug=False)

    x_d = nc.dram_tensor("x", [NS * C, HW], F32, kind="ExternalInput").ap()
    w1_d = nc.dram_tensor("w1", [C, HID], F32, kind="ExternalInput").ap()
    b1_d = nc.dram_tensor("b1", [HID, 1], F32, kind="ExternalInput").ap()
    w2_d = nc.dram_tensor("w2", [HID, C * D], F32, kind="ExternalInput").ap()
    b2_d = nc.dram_tensor("b2", [1, C * D], F32, kind="ExternalInput").ap()
    ident_d = nc.dram_tensor("ident", [P, P], F32, kind="ExternalInput").ap()
    nofs_d = nc.dram_tensor("nofs", [P, 2], F32, kind="ExternalInput").ap()
    out_d = nc.dram_tensor("out", [NS * C, HW], F32, kind="ExternalOutput").ap()

    with tile.TileContext(nc) as tc:
        with (
            tc.tile_pool(name="xin", bufs=3) as xin_pool,
            tc.tile_pool(name="gbuf", bufs=3) as g_pool,
            tc.tile_pool(name="small", bufs=1) as sm,
            tc.tile_pool(name="psum", bufs=1, space="PSUM") as psum,
        ):
            # ---- constants / weights into SBUF (scalar ring; x loads use sync) ----
            w1_sb = sm.tile([P, CT, HID], F32)   # chunk ct = channels ct*128..+128
            nc.scalar.dma_start(
                out=w1_sb[:], in_=w1_d.rearrange("(c p) m -> p c m", p=P)
            )
            w2_sb = sm.tile([HID, C * D], F32)
            nc.scalar.dma_start(out=w2_sb[:], in_=w2_d)
            b1_sb = sm.tile([HID, 1], F32)
            nc.scalar.dma_start(out=b1_sb[:], in_=b1_d)
            b2_sb = sm.tile([1, C * D], F32)
            nc.scalar.dma_start(out=b2_sb[:], in_=b2_d)
            ident_sb = sm.tile([P, P], F32)
            nc.scalar.dma_start(out=ident_sb[:], in_=ident_d)
            nofs_sb = sm.tile([P, 2], F32)
            nc.scalar.dma_start(out=nofs_sb[:], in_=nofs_d)
            twos_sb = sm.tile([1, P], F32)
            nc.vector.memset(twos_sb[:], 2.0)

            # pooling accumulators: [P, ct, n] -> matmul rhs slice = [K=128, cols]
            avgp = sm.tile([P, CT, NS], F32)
            maxp = sm.tile([P, CT, NS], F32)
            scratch = sm.tile([P, HW], F32)

            # per-pair topk tiles (rows at partition 32*d + n; rest zeroed)
            vals = [[sm.tile([P, C], F32, name=f"vals{pp}_{i}") for i in range(2)]
                    for pp in range(2)]
            for pp in range(2):
                for i in range(2):
                    nc.gpsimd.memset(vals[pp][i][:], 0.0)

            def load_and_pool(n):
                for ct in range(CT):
                    row0 = (n * CT + ct) * P
                    xt = xin_pool.tile([P, HW], F32, tag="xt")
                    nc.sync.dma_start(out=xt[:], in_=x_d[row0 : row0 + P, :])
                    # avg pool on ScalarE: accum_out sums copy(x * 1/HW)
                    nc.scalar.activation(
                        out=scratch[:],
                        in_=xt[:],
                        func=mybir.ActivationFunctionType.Copy,
                        scale=1.0 / HW,
                        accum_out=avgp[:, ct, n : n + 1],
                    )
                    nc.vector.reduce_max(
                        out=maxp[:, ct, n : n + 1], in_=xt[:], axis=mybir.AxisListType.X
                    )

            def mlp_pair(pp):
                """MLP for samples {2pp, 2pp+1}: psum py rows 32d+n = y[n, :]."""
                ns = slice(2 * pp, 2 * pp + 2)
                hw_wides = []
                for name, pool_t in (("a", avgp), ("m", maxp)):
                    ph = psum.tile([HID, 2], F32, space="PSUM", tag="ph")
                    for ct in range(CT):
                        nc.tensor.matmul(
                            out=ph[:],
                            lhsT=w1_sb[:, ct, :],
                            rhs=pool_t[:, ct, ns],
                            start=(ct == 0),
                            stop=(ct == CT - 1),
                        )
                    hT = sm.tile([HID, 2], F32, name=f"hT_{name}{pp}")
                    nc.scalar.activation(
                        out=hT[:],
                        in_=ph[:],
                        func=mybir.ActivationFunctionType.Relu,
                        bias=b1_sb[:, :],
                    )
                    # replicate to cols 32d + n so matmul writes partition 32d+n
                    hw_t = sm.tile([HID, P], F32, name=f"hw_{name}{pp}")
                    nc.gpsimd.memset(hw_t[:], 0.0)
                    for d in range(D):
                        nc.vector.tensor_copy(
                            out=hw_t[:, 32 * d : 32 * d + 2], in_=hT[:]
                        )
                    hw_wides.append(hw_t)

                py = psum.tile([P, C * D], F32, space="PSUM", tag="py")
                for s in range(C * D // 512):
                    sl = slice(s * 512, (s + 1) * 512)
                    nc.tensor.matmul(
                        out=py[:, sl], lhsT=hw_wides[0][:], rhs=w2_sb[:, sl],
                        start=True, stop=False,
                    )
                    nc.tensor.matmul(
                        out=py[:, sl], lhsT=hw_wides[1][:], rhs=w2_sb[:, sl],
                        start=False, stop=False,
                    )
                    nc.tensor.matmul(
                        out=py[:, sl], lhsT=twos_sb[:], rhs=b2_sb[:, sl],
                        start=False, stop=True,
                    )
                # vals[32d+n, c] = y[n, c*D + d]
                va = vals[pp][0]
                for d in range(D):
                    nc.vector.tensor_copy(
                        out=va[32 * d : 32 * d + 2, :],
                        in_=py[32 * d : 32 * d + 2, d :: D],
                    )

            def topk_pair(pp):
                """Returns idxT tile: column 32d+n holds topk row (d, n) + n*512."""
                topk_idx = sm.tile([P, BLOCK], U32, name=f"tki{pp}")
                maxv = sm.tile([P, 8], F32, name=f"maxv{pp}")
                cur, nxt = vals[pp]
                for k in range(BLOCK // 8):
                    nc.vector.max(out=maxv[:], in_=cur[:])
                    nc.vector.max_index(
                        out=topk_idx[:, 8 * k : 8 * k + 8],
                        in_max=maxv[:],
                        in_values=cur[:],
                    )
                    if k < BLOCK // 8 - 1:
                        nc.vector.match_replace(
                            out=nxt[:], in_to_replace=maxv[:], in_values=cur[:],
                            imm_value=NEG_FILL,
                        )
                        cur, nxt = nxt, cur

                idx_f = sm.tile([P, BLOCK], F32, name=f"idxf{pp}")
                nc.vector.tensor_copy(out=idx_f[:], in_=topk_idx[:])
                nc.vector.tensor_scalar_add(
                    idx_f[:], idx_f[:], nofs_sb[:, pp : pp + 1]
                )
                pt = psum.tile([P, P], F32, space="PSUM", tag="pt")
                nc.tensor.transpose(out=pt[:], in_=idx_f[:], identity=ident_sb[:])
                idxT = sm.tile([P, P], U32, name=f"idxT{pp}")
                nc.vector.tensor_copy(out=idxT[:], in_=pt[:])
                return idxT

            def gather_pair(pp, idxT):
                for i, n in enumerate((2 * pp, 2 * pp + 1)):
                    for d in range(D):
                        g = g_pool.tile([P, HW], F32, tag="g")
                        nc.gpsimd.indirect_dma_start(
                            out=g[:],
                            out_offset=None,
                            in_=x_d[:, :],
                            in_offset=bass.IndirectOffsetOnAxis(
                                ap=idxT[:, 32 * d + i : 32 * d + i + 1], axis=0
                            ),
                        )
                        o0 = n * C + d * BLOCK
                        nc.scalar.dma_start(out=out_d[o0 : o0 + BLOCK, :], in_=g[:])

            # emission order sets scheduler priority: pair 0 chain first, so
            # pair 1's loads/MLP/topk overlap pair 0's gather+store phase.
            for n in (0, 1):
                load_and_pool(n)
            mlp_pair(0)
            for n in (2, 3):
                load_and_pool(n)
            idxT0 = topk_pair(0)
            gather_pair(0, idxT0)
            mlp_pair(1)
            idxT1 = topk_pair(1)
            gather_pair(1, idxT1)

    nc.compile()
    return nc


_NC_CACHE = None


def _get_nc():
    global _NC_CACHE
    if _NC_CACHE is None:
        _NC_CACHE = _build_program()
    return _NC_CACHE


def _make_in_maps(x, W1, b1, W2, b2):
    x = np.ascontiguousarray(np.asarray(x, dtype=np.float32)).reshape(N_FULL, C, HW)
    W1 = np.asarray(W1, dtype=np.float32)
    b1 = np.asarray(b1, dtype=np.float32).reshape(HID, 1)
    W2 = np.asarray(W2, dtype=np.float32)
    b2 = np.asarray(b2, dtype=np.float32).reshape(1, C * D)
    ident = np.eye(P, dtype=np.float32)
    # partition 32d + i -> topk row (d, n=2*pair+i): DRAM row base = n*512
    pidx = np.arange(P)
    nofs = np.zeros((P, 2), np.float32)
    for pp in range(2):
        nofs[:, pp] = np.where(pidx % 32 < 2, (2 * pp + pidx % 32) * C, 0)
    in_maps = []
    for core in range(N_CORES):
        shard = x[core * NS : (core + 1) * NS].reshape(NS * C, HW)
        in_maps.append(
            {
                "x": np.ascontiguousarray(shard),
                "w1": W1,
                "b1": b1,
                "w2": W2,
                "b2": b2,
                "ident": ident,
                "nofs": nofs,
            }
        )
    return in_maps


def run(inputs, trace=False, **kwargs):
    """Run the SPMD kernel; returns (full_output, BassKernelResults)."""
    nc = _get_nc()
    in_maps = _make_in_maps(
        inputs["x"], inputs["W1"], inputs["b1"], inputs["W2"], inputs["b2"]
    )
    res = run_bass_kernel_spmd(
        nc, in_maps, core_ids=list(range(N_CORES)), trace=trace, **kwargs
    )
    parts = [res.results[i]["out"].reshape(NS, C, 64, 64) for i in range(N_CORES)]
    out = np.concatenate(parts, axis=0)
    return out, res


def kernel(**inputs) -> np.ndarray:
    out, _ = run(inputs)
    return out
